# revision 14
# baseline (speedup 1.0000x reference)
"""SAGAN-style attention block (nn_AttentionBlock) on 8 Trainium2 NeuronCores.

Math (per batch b):
    q = wq @ x + bq            [C8, N]
    k = wk @ x + bk            [C8, N]
    v = wv @ x + bv            [C,  N]
    S[n, m]  = sum_o q[o,n] k[o,m]
    attn     = softmax_m(S)
    out[c,n] = sum_m v[c,m] attn[n,m]
    y        = gamma * out + x

Sharding: 8 cores = 4 batches x 2 halves of the n (query-row) axis.

v2 design notes (all chosen from trace evidence on v1):
  - whole PE path in bf16: host casts x to bf16 (fp32 x kept only for the
    residual add), weights bf16.  fp32(HIGH) matmuls self-load weights and
    cost ~2.4x.
  - wq/wk are host-tiled 4x across PE row-groups (wkt4[c, 32g+o] = wk[o,c])
    so the projection itself replicates k/q into all four 32-row partition
    groups: the quadrant-packed QK^T matmuls (tile_position=(32g,0), which
    run concurrently on the PE) then slice k_rep/q_rep directly - no
    SBUF->SBUF packing DMAs at all.
  - gamma is folded into wv on the host; gamma*bv is folded into the xq
    residual input on the host.  v-projection bias work disappears.
  - q/k biases ride the projection matmul as an extra rank-1 accumulation
    (lhsT = bias row [1,128], rhs = ones row) - zero DVE cost.
  - softmax denominator: per-slot bf16 running sum on DVE (one [128,2048]
    2x-mode add per slot), folded 4->1 on DVE, partition-summed AND
    broadcast in one gpsimd.partition_all_reduce, inverted with the fast
    custom-DVE reciprocal (the [1,512] iterative reciprocal in v1 cost 4us
    per block and stalled the PE at every block boundary).
  - PSUM: 4 banks S^T (single buffer) + 2x2 banks for the out accumulators
    (block-alternating), so block nb+1's matmuls start while block nb's
    tail drains.
"""

import sys

sys.path.insert(0, "/opt/trn_rl_repo")

import numpy as np  # noqa: E402

B, C, HH, WW = 4, 256, 64, 64
N = HH * WW  # 4096
C8 = C // 8  # 32
P = 128
CT = C // P  # 2 channel tiles
NQ = N // 2  # 2048 query rows per core
NBLK = 512  # n-block (query columns per block)
NBLKS = NQ // NBLK  # 4
MT = N // P  # 32 m-tiles (key/value positions)
GRP = 4  # m-tiles per S^T psum slot
NSLOT = MT // GRP  # 8 slots per block
CHUNK = 512
NCHUNKS = N // CHUNK  # 8
QCHUNKS = NQ // CHUNK  # 4
NCORES = 8

_prog = None


def _build(debug_taps=False):
    import concourse.bacc as bacc
    import concourse.bass_isa as bass_isa
    import concourse.mybir as mybir
    import concourse.tile as tile

    f32 = mybir.dt.float32
    f16 = mybir.dt.float16
    bf16 = mybir.dt.bfloat16
    AluAdd = mybir.AluOpType.add
    Exp = mybir.ActivationFunctionType.Exp
    RAdd = bass_isa.ReduceOp.add

    nc = bacc.Bacc("TRN2", target_bir_lowering=False, debug=False)

    dbg = {}
    if debug_taps:
        dbg["k"] = nc.dram_tensor("dbg_k", [P, N], f16, kind="ExternalOutput")
        dbg["q"] = nc.dram_tensor("dbg_q", [P, NQ], f16, kind="ExternalOutput")
        dbg["vt"] = nc.dram_tensor("dbg_vt", [P, MT * C], bf16, kind="ExternalOutput")
        dbg["pt"] = nc.dram_tensor("dbg_pt", [P, GRP * NBLK], bf16, kind="ExternalOutput")
        dbg["dacc"] = nc.dram_tensor("dbg_dacc", [P, GRP * NBLK], bf16, kind="ExternalOutput")
        dbg["dbc"] = nc.dram_tensor("dbg_dbc", [P, NBLK], f32, kind="ExternalOutput")
        dbg["rec"] = nc.dram_tensor("dbg_rec", [P, NBLK], f32, kind="ExternalOutput")
        dbg["acc0"] = nc.dram_tensor("dbg_acc0", [P, NBLK], f32, kind="ExternalOutput")

    xh_d = nc.dram_tensor("xh", [C, N], f16, kind="ExternalInput")
    xqh_d = nc.dram_tensor("xqh", [C, NQ], f16, kind="ExternalInput")
    xq_d = nc.dram_tensor("xq", [C, NQ], f32, kind="ExternalInput")
    wqt4_d = nc.dram_tensor("wqt4", [C, P], f16, kind="ExternalInput")
    wkt4_d = nc.dram_tensor("wkt4", [C, P], f16, kind="ExternalInput")
    wvt_d = nc.dram_tensor("wvt", [C, C], f16, kind="ExternalInput")
    bq4_d = nc.dram_tensor("bq4", [P], f32, kind="ExternalInput")
    bk4_d = nc.dram_tensor("bk4", [P], f32, kind="ExternalInput")
    out_d = nc.dram_tensor("out", [C, NQ], f32, kind="ExternalOutput")

    with tile.TileContext(nc) as tc:
        with (
            tc.tile_pool(name="const", bufs=1) as const,
            tc.tile_pool(name="big", bufs=1) as big,
        ):
            xh = big.tile([P, CT, N], f16)
            xqh = big.tile([P, CT, NQ], f16)
            xq = big.tile([P, CT, NQ], f32)
            k_rep = big.tile([P, N], f16)  # k replicated in 4 row groups
            q_rep = big.tile([P, NQ], f16)
            vt = big.tile([P, MT, C], bf16)  # v^T tiles [m, c], gamma-scaled

            wqt4 = const.tile([P, CT, P], f16)
            wkt4 = const.tile([P, CT, P], f16)
            wvt = const.tile([P, CT, C], f16)
            bq4 = const.tile([P, 1], f32)
            bk4 = const.tile([P, 1], f32)

            nc.sync.dma_start(out=wqt4, in_=wqt4_d.ap().rearrange("(t p) o -> p t o", p=P))
            nc.sync.dma_start(out=wkt4, in_=wkt4_d.ap().rearrange("(t p) o -> p t o", p=P))
            nc.sync.dma_start(out=wvt, in_=wvt_d.ap().rearrange("(t p) o -> p t o", p=P))
            nc.sync.dma_start(out=bq4, in_=bq4_d.ap()[:, None])
            nc.sync.dma_start(out=bk4, in_=bk4_d.ap()[:, None])

            xh_r = xh_d.ap().rearrange("(t p) n -> p t n", p=P)
            xqh_r = xqh_d.ap().rearrange("(t p) n -> p t n", p=P)
            xq_r = xq_d.ap().rearrange("(t p) n -> p t n", p=P)
            out_r = out_d.ap().rearrange("(t p) n -> p t n", p=P)

            # xh in 4 big chunks on the sync queue; xqh in parallel on the
            # vector queue.  The fp32 residual xq is NOT loaded here: its 2MB
            # would compete with the startup-critical loads, and it is first
            # needed only at block 0's tail - its chunk DMAs are emitted
            # inside the block loop below (vector queue) instead.
            for ch in range(4):
                sl = slice(ch * (N // 4), (ch + 1) * (N // 4))
                nc.sync.dma_start(out=xh[:, :, sl], in_=xh_r[:, :, sl])
            for ch in range(2):
                sl = slice(ch * (NQ // 2), (ch + 1) * (NQ // 2))
                nc.scalar.dma_start(out=xqh[:, :, sl], in_=xqh_r[:, :, sl])

            # ---- phase A: q/k/v projections (all bf16 on the PE) ----
            with tc.tile_pool(name="pa", bufs=2, space="PSUM") as pap:
                def proj_chunk(dst, w4, bcol, src_x, sl):
                    pp = pap.tile([P, CHUNK], f32, tag="pj", name="pp")
                    for t in range(CT):
                        nc.tensor.matmul(
                            pp, lhsT=w4[:, t, :], rhs=src_x[:, t, sl],
                            start=(t == 0), stop=(t == CT - 1),
                        )
                    # fused drain + per-partition bias add + f16 cast
                    nc.vector.tensor_scalar_add(dst, pp, bcol)

                for ch in range(NCHUNKS):
                    sl = slice(ch * CHUNK, (ch + 1) * CHUNK)
                    proj_chunk(k_rep[:, sl], wkt4, bk4, xh, sl)
                for ch in range(QCHUNKS):
                    sl = slice(ch * CHUNK, (ch + 1) * CHUNK)
                    proj_chunk(q_rep[:, sl], wqt4, bq4, xqh, sl)
                for mt in range(MT):
                    msl = slice(mt * P, (mt + 1) * P)
                    vp = pap.tile([P, CHUNK], f32, tag="pj", name="vp")
                    for t in range(CT):
                        nc.tensor.matmul(
                            vp[:, :C], lhsT=xh[:, t, msl], rhs=wvt[:, t, :],
                            start=(t == 0), stop=(t == CT - 1),
                        )
                    nc.vector.tensor_copy(out=vt[:, mt, :], in_=vp[:, :C])
                if debug_taps:
                    nc.sync.dma_start(out=dbg["k"].ap(), in_=k_rep)
                    nc.sync.dma_start(out=dbg["q"].ap(), in_=q_rep)
                    nc.sync.dma_start(out=dbg["vt"].ap().rearrange("p (m c) -> p m c", m=MT), in_=vt)

            # ---- phase B: attention ----
            with (
                tc.tile_pool(name="st_ps", bufs=1, space="PSUM") as stp,
                tc.tile_pool(name="acc_ps", bufs=2, space="PSUM") as accp,
                tc.tile_pool(name="ptp", bufs=3) as ptp,
                tc.tile_pool(name="dap", bufs=2) as dap,
                tc.tile_pool(name="dnp", bufs=2) as dnp,
                tc.tile_pool(name="finp", bufs=4) as finp,
            ):
                bstate = {}

                def emit_av(nb, mg, pt):
                    accs, dacc, nsl = bstate[nb]
                    for i in range(GRP):
                        mt = GRP * mg + i
                        for cc in range(CT):
                            nc.tensor.matmul(
                                accs[cc],
                                lhsT=vt[:, mt, cc * P:(cc + 1) * P],
                                rhs=pt[:, i, :],
                                start=(mt == 0),
                                stop=(mt == MT - 1),
                            )
                    # denominator partial: one 2048-elem bf16 add per slot
                    if mg == 0:
                        nc.vector.tensor_copy(out=dacc, in_=pt)
                    else:
                        nc.vector.tensor_tensor(dacc, dacc, pt, AluAdd)

                def emit_tail(nb):
                    accs, dacc, nsl = bstate.pop(nb)
                    d2 = dnp.tile([P, 2, NBLK], bf16, tag="d2", name="d2")
                    nc.vector.tensor_tensor(d2, dacc[:, 0:2, :], dacc[:, 2:4, :], AluAdd)
                    d1 = dnp.tile([P, NBLK], bf16, tag="d1", name="d1")
                    nc.vector.tensor_tensor(d1, d2[:, 0, :], d2[:, 1, :], AluAdd)
                    # sum over partitions, result broadcast to all partitions
                    dbc = dnp.tile([P, NBLK], f32, tag="dbc", name="dbc")
                    nc.gpsimd.partition_all_reduce(dbc, d1, channels=P, reduce_op=RAdd)
                    rec = dnp.tile([P, NBLK], f32, tag="rec", name="rec")
                    nc.vector.reciprocal_approx_fast(rec, dbc)
                    if debug_taps and nb == 0:
                        nc.sync.dma_start(out=dbg["dacc"].ap().rearrange("p (g n) -> p g n", g=GRP), in_=dacc)
                        nc.sync.dma_start(out=dbg["dbc"].ap(), in_=dbc)
                        nc.sync.dma_start(out=dbg["rec"].ap(), in_=rec)
                        acc_sb = finp.tile([P, NBLK], f32, tag="fin", name="accsb")
                        nc.vector.tensor_copy(out=acc_sb, in_=accs[0])
                        nc.sync.dma_start(out=dbg["acc0"].ap(), in_=acc_sb)
                    for cc in range(CT):
                        fin = finp.tile([P, NBLK], f32, tag="fin", name="fin")
                        nc.vector.tensor_mul(out=fin, in0=accs[cc], in1=rec)
                        nc.vector.tensor_add(out=fin, in0=fin, in1=xq[:, cc, nsl])
                        nc.sync.dma_start(out=out_r[:, cc, nsl], in_=fin)

                prev = None
                for nb in range(NBLKS):
                    nsl = slice(nb * NBLK, (nb + 1) * NBLK)
                    a0 = accp.tile([P, NBLK], f32, tag="o0", name="a0")
                    a1 = accp.tile([P, NBLK], f32, tag="o1", name="a1")
                    dacc = dap.tile([P, GRP, NBLK], bf16, tag="da", name="dacc")
                    bstate[nb] = ([a0, a1], dacc, nsl)
                    # late-load this block's fp32 residual slice
                    nc.gpsimd.dma_start(out=xq[:, :, nsl], in_=xq_r[:, :, nsl])
                    for mg in range(NSLOT):
                        st = stp.tile([P, GRP, NBLK], f32, tag="st", name="st")
                        for g in range(GRP):
                            mt = GRP * mg + g
                            nc.tensor.matmul(
                                st[:, g, :],
                                lhsT=k_rep[32 * g:32 * g + 32, mt * P:(mt + 1) * P],
                                rhs=q_rep[32 * g:32 * g + 32, nsl],
                                start=True,
                                stop=True,
                                tile_position=(32 * g, 0),
                            )
                        pt = ptp.tile([P, GRP, NBLK], bf16, tag="pt", name="pt")
                        nc.scalar.activation(out=pt, in_=st, func=Exp)
                        if debug_taps and nb == 0 and mg == 0:
                            nc.sync.dma_start(out=dbg["pt"].ap().rearrange("p (g n) -> p g n", g=GRP), in_=pt)
                        if prev is not None:
                            pnb, pmg, ppt = prev
                            emit_av(pnb, pmg, ppt)
                            if pmg == NSLOT - 1:
                                emit_tail(pnb)
                        prev = (nb, mg, pt)
                pnb, pmg, ppt = prev
                emit_av(pnb, pmg, ppt)
                emit_tail(pnb)

    nc.compile()
    return nc


def _get_prog():
    global _prog
    if _prog is None:
        _prog = _build()
    return _prog


def make_in_maps(inputs):
    import ml_dtypes

    bf = ml_dtypes.bfloat16
    x = np.ascontiguousarray(inputs["x"], dtype=np.float32).reshape(B, C, N)
    gamma = float(np.asarray(inputs["gamma"], np.float32).reshape(()))
    wq = np.asarray(inputs["wq"], np.float32)
    wk = np.asarray(inputs["wk"], np.float32)
    wv = np.asarray(inputs["wv"], np.float32)
    bq = np.asarray(inputs["bq"], np.float32)
    bk = np.asarray(inputs["bk"], np.float32)
    bv = np.asarray(inputs["bv"], np.float32)

    wqt4 = np.ascontiguousarray(np.tile(wq.T, (1, 4)).astype(np.float16))  # [C,128]
    wkt4 = np.ascontiguousarray(np.tile(wk.T, (1, 4)).astype(np.float16))
    wvt = np.ascontiguousarray((gamma * wv.T).astype(np.float16))  # [C,C]
    bq4 = np.ascontiguousarray(np.tile(bq, 4).astype(np.float32))  # [128]
    bk4 = np.ascontiguousarray(np.tile(bk, 4).astype(np.float32))
    gbv = (gamma * bv).astype(np.float32)[:, None]  # [C,1]

    x_f16 = x.astype(np.float16)  # [B,C,N]
    in_maps = []
    for core in range(NCORES):
        b, h = divmod(core, 2)
        xq = x[b][:, h * NQ:(h + 1) * NQ] + gbv
        in_maps.append(
            {
                "xh": x_f16[b],
                "xqh": np.ascontiguousarray(x_f16[b][:, h * NQ:(h + 1) * NQ]),
                "xq": np.ascontiguousarray(xq, dtype=np.float32),
                "wqt4": wqt4,
                "wkt4": wkt4,
                "wvt": wvt,
                "bq4": bq4,
                "bk4": bk4,
            }
        )
    return in_maps


def assemble(results):
    out = np.empty((B, C, N), np.float32)
    for core in range(NCORES):
        b, h = divmod(core, 2)
        out[b][:, h * NQ:(h + 1) * NQ] = results[core]["out"]
    return out.reshape(B, C, HH, WW)


def kernel(**inputs):
    from concourse.bass_utils import run_bass_kernel_spmd

    nc = _get_prog()
    in_maps = make_in_maps(inputs)
    res = run_bass_kernel_spmd(nc, in_maps, core_ids=list(range(NCORES)))
    return assemble(res.results)


# revision 15
# speedup vs baseline: 1.0120x; 1.0120x over previous
"""SAGAN-style attention block (nn_AttentionBlock) on 8 Trainium2 NeuronCores.

Math (per batch b):
    q = wq @ x + bq            [C8, N]
    k = wk @ x + bk            [C8, N]
    v = wv @ x + bv            [C,  N]
    S[n, m]  = sum_o q[o,n] k[o,m]
    attn     = softmax_m(S)
    out[c,n] = sum_m v[c,m] attn[n,m]
    y        = gamma * out + x

Sharding: 8 cores = 4 batches x 2 halves of the n (query-row) axis.

Design notes (v4, all from trace evidence):
  - q/k path in fp16 (bf16 there costs 1.5e-2 rel err via exp amplification;
    fp16 gives 1.8e-3).  P=exp(S) and v^T in bf16 (P overflows fp16).
  - wq/wk host-tiled 4x across PE row-groups so the projection replicates
    q/k into all four 32-row partition groups; the quadrant-packed QK^T
    matmuls (tile_position=(32g,0), concurrent on the PE) slice them
    directly - no packing DMAs.
  - gamma folded into wv on the host; gamma*bv folded into the xq residual
    input on the host.
  - All big SBUF tensors are split into per-chunk tiles: Tile tracks
    dependencies per tile, so with one monolithic xh tile the first
    k-projection waited for ALL xh DMAs (~24us).  Chunked, the
    QK/exp stream starts as soon as chunk 0 lands.
  - softmax denominator: per-slot [128,2048] bf16 running sum on DVE,
    folded 4->1, partition-summed+broadcast by gpsimd.partition_all_reduce,
    inverted by the fast custom-DVE reciprocal (1-cycle/elem; the [1,512]
    iterative reciprocal cost 4us/block in v1).
  - PSUM: 4 banks S^T (single buffer) + 2x2 accumulator banks alternating
    between blocks so block nb+1's AV matmuls overlap block nb's tail.
  - fp32 residual xq is late-loaded per block (gpsimd queue) so its 2MB
    does not compete with the startup-critical fp16 loads.
"""

import sys

sys.path.insert(0, "/opt/trn_rl_repo")

import numpy as np  # noqa: E402

B, C, HH, WW = 4, 256, 64, 64
N = HH * WW  # 4096
C8 = C // 8  # 32
P = 128
CT = C // P  # 2 channel tiles
NQ = N // 2  # 2048 query rows per core
NBLK = 512  # n-block (query columns per block)
NBLKS = NQ // NBLK  # 4
MT = N // P  # 32 m-tiles (key/value positions)
GRP = 4  # m-tiles per S^T psum slot
NSLOT = MT // GRP  # 8 slots per block
CHUNK = 512
NCHUNKS = N // CHUNK  # 8
QCHUNKS = NQ // CHUNK  # 4
NCORES = 8

_prog = None


def _build(debug_taps=False):
    import concourse.bacc as bacc
    import concourse.bass_isa as bass_isa
    import concourse.mybir as mybir
    import concourse.tile as tile

    f32 = mybir.dt.float32
    f16 = mybir.dt.float16
    bf16 = mybir.dt.bfloat16
    AluAdd = mybir.AluOpType.add
    Exp = mybir.ActivationFunctionType.Exp
    RAdd = bass_isa.ReduceOp.add

    nc = bacc.Bacc("TRN2", target_bir_lowering=False, debug=False)

    dbg = {}
    if debug_taps:
        dbg["dacc"] = nc.dram_tensor("dbg_dacc", [P, GRP * NBLK], bf16, kind="ExternalOutput")
        dbg["dbc"] = nc.dram_tensor("dbg_dbc", [P, NBLK], f32, kind="ExternalOutput")

    xh_d = nc.dram_tensor("xh", [C, N], f16, kind="ExternalInput")
    xqh_d = nc.dram_tensor("xqh", [C, NQ], f16, kind="ExternalInput")
    xq_d = nc.dram_tensor("xq", [C, NQ], f32, kind="ExternalInput")
    wqt4_d = nc.dram_tensor("wqt4", [C, P], f16, kind="ExternalInput")
    wkt4_d = nc.dram_tensor("wkt4", [C, P], f16, kind="ExternalInput")
    wvt_d = nc.dram_tensor("wvt", [C, C], f16, kind="ExternalInput")
    bq4_d = nc.dram_tensor("bq4", [P], f32, kind="ExternalInput")
    bk4_d = nc.dram_tensor("bk4", [P], f32, kind="ExternalInput")
    out_d = nc.dram_tensor("out", [C, NQ], f32, kind="ExternalOutput")

    with tile.TileContext(nc) as tc:
        with (
            tc.tile_pool(name="const", bufs=1) as const,
            tc.tile_pool(name="big", bufs=1) as big,
        ):
            # per-chunk tiles => fine-grained DMA->compute dependencies
            xh_c = [big.tile([P, CT, CHUNK], f16, name=f"xh{i}") for i in range(NCHUNKS)]
            xqh_c = [big.tile([P, CT, CHUNK], f16, name=f"xqh{i}") for i in range(QCHUNKS)]
            xq_c = [big.tile([P, CT, NBLK], f32, name=f"xq{i}") for i in range(NBLKS)]
            k_c = [big.tile([P, CHUNK], f16, name=f"k{i}") for i in range(NCHUNKS)]
            q_c = [big.tile([P, NBLK], f16, name=f"q{i}") for i in range(NBLKS)]
            vt_c = [big.tile([P, GRP, C], bf16, name=f"vt{i}") for i in range(NSLOT)]

            wqt4 = const.tile([P, CT, P], f16)
            wkt4 = const.tile([P, CT, P], f16)
            wvt = const.tile([P, CT, C], f16)
            bq4 = const.tile([P, 1], f32)
            bk4 = const.tile([P, 1], f32)

            xh_r = xh_d.ap().rearrange("(t p) n -> p t n", p=P)
            xqh_r = xqh_d.ap().rearrange("(t p) n -> p t n", p=P)
            xq_r = xq_d.ap().rearrange("(t p) n -> p t n", p=P)
            out_r = out_d.ap().rearrange("(t p) n -> p t n", p=P)

            # sync queue: k-path weights then xh chunks (k-proj critical path)
            nc.sync.dma_start(out=wkt4, in_=wkt4_d.ap().rearrange("(t p) o -> p t o", p=P))
            nc.sync.dma_start(out=bk4, in_=bk4_d.ap()[:, None])
            for i in range(NCHUNKS):
                sl = slice(i * CHUNK, (i + 1) * CHUNK)
                nc.sync.dma_start(out=xh_c[i], in_=xh_r[:, :, sl])
            # scalar queue in parallel: q-path weights, xqh, v weights
            nc.scalar.dma_start(out=wqt4, in_=wqt4_d.ap().rearrange("(t p) o -> p t o", p=P))
            nc.scalar.dma_start(out=bq4, in_=bq4_d.ap()[:, None])
            nc.scalar.dma_start(out=xqh_c[0], in_=xqh_r[:, :, 0:CHUNK])
            nc.scalar.dma_start(out=wvt, in_=wvt_d.ap().rearrange("(t p) o -> p t o", p=P))
            for i in range(1, QCHUNKS):
                sl = slice(i * CHUNK, (i + 1) * CHUNK)
                nc.scalar.dma_start(out=xqh_c[i], in_=xqh_r[:, :, sl])

            # ---- phase A: q/k/v projections (fp16 on the PE) ----
            with tc.tile_pool(name="pa", bufs=2, space="PSUM") as pap:
                def proj_chunk(dst, w4, bcol, src, name):
                    pp = pap.tile([P, CHUNK], f32, tag="pj", name=name)
                    for t in range(CT):
                        nc.tensor.matmul(
                            pp, lhsT=w4[:, t, :], rhs=src[:, t, :],
                            start=(t == 0), stop=(t == CT - 1),
                        )
                    # fused drain + per-partition bias add + f16 cast
                    nc.vector.tensor_scalar_add(dst, pp, bcol)

                for ch in range(NCHUNKS):
                    proj_chunk(k_c[ch], wkt4, bk4, xh_c[ch], "kp")
                for ch in range(QCHUNKS):
                    proj_chunk(q_c[ch], wqt4, bq4, xqh_c[ch], "qp")
                for mt in range(MT):
                    src = xh_c[mt // 4]
                    msl = slice((mt % 4) * P, (mt % 4 + 1) * P)
                    vp = pap.tile([P, CHUNK], f32, tag="pj", name="vp")
                    for t in range(CT):
                        nc.tensor.matmul(
                            vp[:, :C], lhsT=src[:, t, msl], rhs=wvt[:, t, :],
                            start=(t == 0), stop=(t == CT - 1),
                        )
                    nc.vector.tensor_copy(out=vt_c[mt // 4][:, mt % 4, :], in_=vp[:, :C])

            # ---- phase B: attention ----
            with (
                tc.tile_pool(name="st_ps", bufs=1, space="PSUM") as stp,
                tc.tile_pool(name="acc_ps", bufs=2, space="PSUM") as accp,
                tc.tile_pool(name="ptp", bufs=3) as ptp,
                tc.tile_pool(name="dap", bufs=2) as dap,
                tc.tile_pool(name="dnp", bufs=2) as dnp,
                tc.tile_pool(name="finp", bufs=4) as finp,
            ):
                bstate = {}

                def emit_av(nb, mg, pt):
                    accs, dacc, nb_ = bstate[nb]
                    for i in range(GRP):
                        mt = GRP * mg + i
                        for cc in range(CT):
                            nc.tensor.matmul(
                                accs[cc],
                                lhsT=vt_c[mg][:, i, cc * P:(cc + 1) * P],
                                rhs=pt[:, i, :],
                                start=(mt == 0),
                                stop=(mt == MT - 1),
                            )
                    # denominator partial: one 2048-elem bf16 add per slot
                    if mg == 0:
                        nc.vector.tensor_copy(out=dacc, in_=pt)
                    else:
                        nc.vector.tensor_tensor(dacc, dacc, pt, AluAdd)

                def emit_tail(nb):
                    accs, dacc, nb_ = bstate.pop(nb)
                    nsl = slice(nb * NBLK, (nb + 1) * NBLK)
                    d2 = dnp.tile([P, 2, NBLK], bf16, tag="d2", name="d2")
                    nc.vector.tensor_tensor(d2, dacc[:, 0:2, :], dacc[:, 2:4, :], AluAdd)
                    d1 = dnp.tile([P, NBLK], bf16, tag="d1", name="d1")
                    nc.vector.tensor_tensor(d1, d2[:, 0, :], d2[:, 1, :], AluAdd)
                    # sum over partitions, result broadcast to all partitions
                    dbc = dnp.tile([P, NBLK], f32, tag="dbc", name="dbc")
                    nc.gpsimd.partition_all_reduce(dbc, d1, channels=P, reduce_op=RAdd)
                    rec = dnp.tile([P, NBLK], f32, tag="rec", name="rec")
                    nc.vector.reciprocal_approx_fast(rec, dbc)
                    if debug_taps and nb == 0:
                        nc.sync.dma_start(out=dbg["dacc"].ap().rearrange("p (g n) -> p g n", g=GRP), in_=dacc)
                        nc.sync.dma_start(out=dbg["dbc"].ap(), in_=dbc)
                    for cc in range(CT):
                        fin = finp.tile([P, NBLK], f32, tag="fin", name="fin")
                        nc.vector.tensor_mul(out=fin, in0=accs[cc], in1=rec)
                        nc.vector.tensor_add(out=fin, in0=fin, in1=xq_c[nb_][:, cc, :])
                        nc.sync.dma_start(out=out_r[:, cc, nsl], in_=fin)

                prev = None
                for nb in range(NBLKS):
                    a0 = accp.tile([P, NBLK], f32, tag="o0", name="a0")
                    a1 = accp.tile([P, NBLK], f32, tag="o1", name="a1")
                    dacc = dap.tile([P, GRP, NBLK], bf16, tag="da", name="dacc")
                    bstate[nb] = ([a0, a1], dacc, nb)
                    # late-load this block's fp32 residual slice
                    nsl = slice(nb * NBLK, (nb + 1) * NBLK)
                    nc.gpsimd.dma_start(out=xq_c[nb], in_=xq_r[:, :, nsl])
                    for mg in range(NSLOT):
                        st = stp.tile([P, GRP, NBLK], f32, tag="st", name="st")
                        for g in range(GRP):
                            nc.tensor.matmul(
                                st[:, g, :],
                                lhsT=k_c[mg][32 * g:32 * g + 32, g * P:(g + 1) * P],
                                rhs=q_c[nb][32 * g:32 * g + 32, :],
                                start=True,
                                stop=True,
                                tile_position=(32 * g, 0),
                            )
                        pt = ptp.tile([P, GRP, NBLK], bf16, tag="pt", name="pt")
                        nc.scalar.activation(out=pt, in_=st, func=Exp)
                        if prev is not None:
                            pnb, pmg, ppt = prev
                            emit_av(pnb, pmg, ppt)
                            if pmg == NSLOT - 1:
                                emit_tail(pnb)
                        prev = (nb, mg, pt)
                pnb, pmg, ppt = prev
                emit_av(pnb, pmg, ppt)
                emit_tail(pnb)

    nc.compile()
    return nc


def _get_prog():
    global _prog
    if _prog is None:
        _prog = _build()
    return _prog


def make_in_maps(inputs):
    x = np.ascontiguousarray(inputs["x"], dtype=np.float32).reshape(B, C, N)
    gamma = float(np.asarray(inputs["gamma"], np.float32).reshape(()))
    wq = np.asarray(inputs["wq"], np.float32)
    wk = np.asarray(inputs["wk"], np.float32)
    wv = np.asarray(inputs["wv"], np.float32)
    bq = np.asarray(inputs["bq"], np.float32)
    bk = np.asarray(inputs["bk"], np.float32)
    bv = np.asarray(inputs["bv"], np.float32)

    wqt4 = np.ascontiguousarray(np.tile(wq.T, (1, 4)).astype(np.float16))  # [C,128]
    wkt4 = np.ascontiguousarray(np.tile(wk.T, (1, 4)).astype(np.float16))
    wvt = np.ascontiguousarray((gamma * wv.T).astype(np.float16))  # [C,C]
    bq4 = np.ascontiguousarray(np.tile(bq, 4).astype(np.float32))  # [128]
    bk4 = np.ascontiguousarray(np.tile(bk, 4).astype(np.float32))
    gbv = (gamma * bv).astype(np.float32)[:, None]  # [C,1]

    x_f16 = x.astype(np.float16)  # [B,C,N]
    in_maps = []
    for core in range(NCORES):
        b, h = divmod(core, 2)
        xq = x[b][:, h * NQ:(h + 1) * NQ] + gbv
        in_maps.append(
            {
                "xh": x_f16[b],
                "xqh": np.ascontiguousarray(x_f16[b][:, h * NQ:(h + 1) * NQ]),
                "xq": np.ascontiguousarray(xq, dtype=np.float32),
                "wqt4": wqt4,
                "wkt4": wkt4,
                "wvt": wvt,
                "bq4": bq4,
                "bk4": bk4,
            }
        )
    return in_maps


def assemble(results):
    out = np.empty((B, C, N), np.float32)
    for core in range(NCORES):
        b, h = divmod(core, 2)
        out[b][:, h * NQ:(h + 1) * NQ] = results[core]["out"]
    return out.reshape(B, C, HH, WW)


def kernel(**inputs):
    from concourse.bass_utils import run_bass_kernel_spmd

    nc = _get_prog()
    in_maps = make_in_maps(inputs)
    res = run_bass_kernel_spmd(nc, in_maps, core_ids=list(range(NCORES)))
    return assemble(res.results)


# revision 16
# speedup vs baseline: 1.0286x; 1.0164x over previous
"""SAGAN-style attention block (nn_AttentionBlock) on 8 Trainium2 NeuronCores.

Math (per batch b):
    q = wq @ x + bq            [C8, N]
    k = wk @ x + bk            [C8, N]
    v = wv @ x + bv            [C,  N]
    S[n, m]  = sum_o q[o,n] k[o,m]
    attn     = softmax_m(S)
    out[c,n] = sum_m v[c,m] attn[n,m]
    y        = gamma * out + x

Sharding: 8 cores = 4 batches x 2 halves of the n (query-row) axis.

Design notes (v4, all from trace evidence):
  - q/k path in fp16 (bf16 there costs 1.5e-2 rel err via exp amplification;
    fp16 gives 1.8e-3).  P=exp(S) and v^T in bf16 (P overflows fp16).
  - wq/wk host-tiled 4x across PE row-groups so the projection replicates
    q/k into all four 32-row partition groups; the quadrant-packed QK^T
    matmuls (tile_position=(32g,0), concurrent on the PE) slice them
    directly - no packing DMAs.
  - gamma folded into wv on the host; gamma*bv folded into the xq residual
    input on the host.
  - All big SBUF tensors are split into per-chunk tiles: Tile tracks
    dependencies per tile, so with one monolithic xh tile the first
    k-projection waited for ALL xh DMAs (~24us).  Chunked, the
    QK/exp stream starts as soon as chunk 0 lands.
  - softmax denominator: per-slot [128,2048] bf16 running sum on DVE,
    folded 4->1, partition-summed+broadcast by gpsimd.partition_all_reduce,
    inverted by the fast custom-DVE reciprocal (1-cycle/elem; the [1,512]
    iterative reciprocal cost 4us/block in v1).
  - PSUM: 4 banks S^T (single buffer) + 2x2 accumulator banks alternating
    between blocks so block nb+1's AV matmuls overlap block nb's tail.
  - fp32 residual xq is late-loaded per block (gpsimd queue) so its 2MB
    does not compete with the startup-critical fp16 loads.
"""

import sys

sys.path.insert(0, "/opt/trn_rl_repo")

import numpy as np  # noqa: E402

B, C, HH, WW = 4, 256, 64, 64
N = HH * WW  # 4096
C8 = C // 8  # 32
P = 128
CT = C // P  # 2 channel tiles
NQ = N // 2  # 2048 query rows per core
NBLK = 512  # n-block (query columns per block)
NBLKS = NQ // NBLK  # 4
MT = N // P  # 32 m-tiles (key/value positions)
GRP = 4  # m-tiles per S^T psum slot
NSLOT = MT // GRP  # 8 slots per block
CHUNK = 512
NCHUNKS = N // CHUNK  # 8
QCHUNKS = NQ // CHUNK  # 4
NCORES = 8

_prog = None


def _build(debug_taps=False):
    import concourse.bacc as bacc
    import concourse.bass_isa as bass_isa
    import concourse.mybir as mybir
    import concourse.tile as tile

    f32 = mybir.dt.float32
    f16 = mybir.dt.float16
    bf16 = mybir.dt.bfloat16
    AluAdd = mybir.AluOpType.add
    Exp = mybir.ActivationFunctionType.Exp
    RAdd = bass_isa.ReduceOp.add

    nc = bacc.Bacc("TRN2", target_bir_lowering=False, debug=False)

    dbg = {}
    if debug_taps:
        dbg["dacc"] = nc.dram_tensor("dbg_dacc", [P, GRP * NBLK], bf16, kind="ExternalOutput")
        dbg["dbc"] = nc.dram_tensor("dbg_dbc", [P, NBLK], f32, kind="ExternalOutput")

    xh_d = nc.dram_tensor("xh", [C, N], f16, kind="ExternalInput")
    xt_d = nc.dram_tensor("xt", [N, C], bf16, kind="ExternalInput")
    xqh_d = nc.dram_tensor("xqh", [C, NQ], f16, kind="ExternalInput")
    xq_d = nc.dram_tensor("xq", [C, NQ], f32, kind="ExternalInput")
    wqt4_d = nc.dram_tensor("wqt4", [C, P], f16, kind="ExternalInput")
    wkt4_d = nc.dram_tensor("wkt4", [C, P], f16, kind="ExternalInput")
    wvt_d = nc.dram_tensor("wvt", [C, C], bf16, kind="ExternalInput")
    bq4_d = nc.dram_tensor("bq4", [P], f32, kind="ExternalInput")
    bk4_d = nc.dram_tensor("bk4", [P], f32, kind="ExternalInput")
    out_d = nc.dram_tensor("out", [C, NQ], f32, kind="ExternalOutput")

    with tile.TileContext(nc) as tc:
        with (
            tc.tile_pool(name="const", bufs=1) as const,
            tc.tile_pool(name="big", bufs=1) as big,
        ):
            # per-chunk tiles => fine-grained DMA->compute dependencies
            xh_c = [big.tile([P, CT, CHUNK], f16, name=f"xh{i}") for i in range(NCHUNKS)]
            xqh_c = [big.tile([P, CT, CHUNK], f16, name=f"xqh{i}") for i in range(QCHUNKS)]
            xq_c = [big.tile([P, CT, NBLK], f32, name=f"xq{i}") for i in range(NBLKS)]
            k_c = [big.tile([P, CHUNK], f16, name=f"k{i}") for i in range(NCHUNKS)]
            q_c = [big.tile([P, NBLK], f16, name=f"q{i}") for i in range(NBLKS)]
            xt_c = [big.tile([P, GRP, C], bf16, name=f"xt{i}") for i in range(NSLOT)]

            wqt4 = const.tile([P, CT, P], f16)
            wkt4 = const.tile([P, CT, P], f16)
            wvt = const.tile([P, CT, C], bf16)
            bq4 = const.tile([P, 1], f32)
            bk4 = const.tile([P, 1], f32)

            xh_r = xh_d.ap().rearrange("(t p) n -> p t n", p=P)
            xt_r = xt_d.ap().rearrange("(m p) c -> p m c", p=P)
            xqh_r = xqh_d.ap().rearrange("(t p) n -> p t n", p=P)
            xq_r = xq_d.ap().rearrange("(t p) n -> p t n", p=P)
            out_r = out_d.ap().rearrange("(t p) n -> p t n", p=P)

            # sync queue: k-path weights, then xh (k-proj) and xt (AV lhsT)
            # chunks interleaved so both pipelines start early
            nc.sync.dma_start(out=wkt4, in_=wkt4_d.ap().rearrange("(t p) o -> p t o", p=P))
            nc.sync.dma_start(out=bk4, in_=bk4_d.ap()[:, None])
            for i in range(NCHUNKS):
                sl = slice(i * CHUNK, (i + 1) * CHUNK)
                nc.sync.dma_start(out=xh_c[i], in_=xh_r[:, :, sl])
                nc.sync.dma_start(out=xt_c[i], in_=xt_r[:, GRP * i:GRP * (i + 1), :])
            # scalar queue in parallel: q-path weights, xqh, v weights
            nc.scalar.dma_start(out=wqt4, in_=wqt4_d.ap().rearrange("(t p) o -> p t o", p=P))
            nc.scalar.dma_start(out=bq4, in_=bq4_d.ap()[:, None])
            nc.scalar.dma_start(out=xqh_c[0], in_=xqh_r[:, :, 0:CHUNK])
            nc.scalar.dma_start(out=wvt, in_=wvt_d.ap().rearrange("(t p) o -> p t o", p=P))
            for i in range(1, QCHUNKS):
                sl = slice(i * CHUNK, (i + 1) * CHUNK)
                nc.scalar.dma_start(out=xqh_c[i], in_=xqh_r[:, :, sl])

            # ---- phase A: q/k/v projections (fp16 on the PE) ----
            with tc.tile_pool(name="pa", bufs=2, space="PSUM") as pap:
                def proj_chunk(dst, w4, bcol, src, name):
                    pp = pap.tile([P, CHUNK], f32, tag="pj", name=name)
                    for t in range(CT):
                        nc.tensor.matmul(
                            pp, lhsT=w4[:, t, :], rhs=src[:, t, :],
                            start=(t == 0), stop=(t == CT - 1),
                        )
                    # fused drain + per-partition bias add + f16 cast
                    nc.vector.tensor_scalar_add(dst, pp, bcol)

                for ch in range(NCHUNKS):
                    proj_chunk(k_c[ch], wkt4, bk4, xh_c[ch], "kp")
                for ch in range(QCHUNKS):
                    proj_chunk(q_c[ch], wqt4, bq4, xqh_c[ch], "qp")

            # ---- phase B: attention ----
            with (
                tc.tile_pool(name="st_ps", bufs=1, space="PSUM") as stp,
                tc.tile_pool(name="acc_ps", bufs=2, space="PSUM") as accp,
                tc.tile_pool(name="ptp", bufs=3) as ptp,
                tc.tile_pool(name="dap", bufs=2) as dap,
                tc.tile_pool(name="dnp", bufs=2) as dnp,
                tc.tile_pool(name="finp", bufs=4) as finp,
            ):
                bstate = {}

                def emit_av(nb, mg, pt):
                    accs, dacc, nb_ = bstate[nb]
                    for i in range(GRP):
                        mt = GRP * mg + i
                        for cc in range(CT):
                            nc.tensor.matmul(
                                accs[cc],
                                lhsT=xt_c[mg][:, i, cc * P:(cc + 1) * P],
                                rhs=pt[:, i, :],
                                start=(mt == 0),
                                stop=(mt == MT - 1),
                            )
                    # denominator partial: one 2048-elem bf16 add per slot
                    if mg == 0:
                        nc.vector.tensor_copy(out=dacc, in_=pt)
                    else:
                        nc.vector.tensor_tensor(dacc, dacc, pt, AluAdd)

                def emit_tail(nb):
                    accs, dacc, nb_ = bstate.pop(nb)
                    nsl = slice(nb * NBLK, (nb + 1) * NBLK)
                    # drain Z = x @ P^T to SBUF, then out = wv_g @ Z reusing
                    # the same accumulator banks (WAR dep via Tile)
                    zsb = finp.tile([P, CT, NBLK], bf16, tag="zsb", name="zsb")
                    for cc in range(CT):
                        nc.vector.tensor_copy(out=zsb[:, cc, :], in_=accs[cc])
                    for co in range(CT):
                        for ci in range(CT):
                            nc.tensor.matmul(
                                accs[co],
                                lhsT=wvt[:, ci, co * P:(co + 1) * P],
                                rhs=zsb[:, ci, :],
                                start=(ci == 0),
                                stop=(ci == CT - 1),
                            )
                    d2 = dnp.tile([P, 2, NBLK], bf16, tag="d2", name="d2")
                    nc.vector.tensor_tensor(d2, dacc[:, 0:2, :], dacc[:, 2:4, :], AluAdd)
                    d1 = dnp.tile([P, NBLK], bf16, tag="d1", name="d1")
                    nc.vector.tensor_tensor(d1, d2[:, 0, :], d2[:, 1, :], AluAdd)
                    # sum over partitions, result broadcast to all partitions
                    dbc = dnp.tile([P, NBLK], f32, tag="dbc", name="dbc")
                    nc.gpsimd.partition_all_reduce(dbc, d1, channels=P, reduce_op=RAdd)
                    rec = dnp.tile([P, NBLK], f32, tag="rec", name="rec")
                    nc.vector.reciprocal_approx_fast(rec, dbc)
                    if debug_taps and nb == 0:
                        nc.sync.dma_start(out=dbg["dacc"].ap().rearrange("p (g n) -> p g n", g=GRP), in_=dacc)
                        nc.sync.dma_start(out=dbg["dbc"].ap(), in_=dbc)
                    for cc in range(CT):
                        fin = finp.tile([P, NBLK], f32, tag="fin", name="fin")
                        nc.vector.tensor_mul(out=fin, in0=accs[cc], in1=rec)
                        nc.vector.tensor_add(out=fin, in0=fin, in1=xq_c[nb_][:, cc, :])
                        nc.sync.dma_start(out=out_r[:, cc, nsl], in_=fin)

                prev = None
                for nb in range(NBLKS):
                    a0 = accp.tile([P, NBLK], f32, tag="o0", name="a0")
                    a1 = accp.tile([P, NBLK], f32, tag="o1", name="a1")
                    dacc = dap.tile([P, GRP, NBLK], bf16, tag="da", name="dacc")
                    bstate[nb] = ([a0, a1], dacc, nb)
                    # late-load this block's fp32 residual slice
                    nsl = slice(nb * NBLK, (nb + 1) * NBLK)
                    nc.gpsimd.dma_start(out=xq_c[nb], in_=xq_r[:, :, nsl])
                    for mg in range(NSLOT):
                        st = stp.tile([P, GRP, NBLK], f32, tag="st", name="st")
                        for g in range(GRP):
                            nc.tensor.matmul(
                                st[:, g, :],
                                lhsT=k_c[mg][32 * g:32 * g + 32, g * P:(g + 1) * P],
                                rhs=q_c[nb][32 * g:32 * g + 32, :],
                                start=True,
                                stop=True,
                                tile_position=(32 * g, 0),
                            )
                        pt = ptp.tile([P, GRP, NBLK], bf16, tag="pt", name="pt")
                        nc.scalar.activation(out=pt, in_=st, func=Exp)
                        if prev is not None:
                            pnb, pmg, ppt = prev
                            emit_av(pnb, pmg, ppt)
                            if pmg == NSLOT - 1:
                                emit_tail(pnb)
                        prev = (nb, mg, pt)
                pnb, pmg, ppt = prev
                emit_av(pnb, pmg, ppt)
                emit_tail(pnb)

    nc.compile()
    return nc


def _get_prog():
    global _prog
    if _prog is None:
        _prog = _build()
    return _prog


def make_in_maps(inputs):
    x = np.ascontiguousarray(inputs["x"], dtype=np.float32).reshape(B, C, N)
    gamma = float(np.asarray(inputs["gamma"], np.float32).reshape(()))
    wq = np.asarray(inputs["wq"], np.float32)
    wk = np.asarray(inputs["wk"], np.float32)
    wv = np.asarray(inputs["wv"], np.float32)
    bq = np.asarray(inputs["bq"], np.float32)
    bk = np.asarray(inputs["bk"], np.float32)
    bv = np.asarray(inputs["bv"], np.float32)

    wqt4 = np.ascontiguousarray(np.tile(wq.T, (1, 4)).astype(np.float16))  # [C,128]
    wkt4 = np.ascontiguousarray(np.tile(wk.T, (1, 4)).astype(np.float16))
    import ml_dtypes

    wvt = np.ascontiguousarray((gamma * wv.T).astype(ml_dtypes.bfloat16))  # [C,C]
    bq4 = np.ascontiguousarray(np.tile(bq, 4).astype(np.float32))  # [128]
    bk4 = np.ascontiguousarray(np.tile(bk, 4).astype(np.float32))
    gbv = (gamma * bv).astype(np.float32)[:, None]  # [C,1]

    x_f16 = x.astype(np.float16)  # [B,C,N]
    x_t = np.ascontiguousarray(x.transpose(0, 2, 1)).astype(ml_dtypes.bfloat16)  # [B,N,C]
    in_maps = []
    for core in range(NCORES):
        b, h = divmod(core, 2)
        xq = x[b][:, h * NQ:(h + 1) * NQ] + gbv
        in_maps.append(
            {
                "xh": x_f16[b],
                "xt": x_t[b],
                "xqh": np.ascontiguousarray(x_f16[b][:, h * NQ:(h + 1) * NQ]),
                "xq": np.ascontiguousarray(xq, dtype=np.float32),
                "wqt4": wqt4,
                "wkt4": wkt4,
                "wvt": wvt,
                "bq4": bq4,
                "bk4": bk4,
            }
        )
    return in_maps


def assemble(results):
    out = np.empty((B, C, N), np.float32)
    for core in range(NCORES):
        b, h = divmod(core, 2)
        out[b][:, h * NQ:(h + 1) * NQ] = results[core]["out"]
    return out.reshape(B, C, HH, WW)


def kernel(**inputs):
    from concourse.bass_utils import run_bass_kernel_spmd

    nc = _get_prog()
    in_maps = make_in_maps(inputs)
    res = run_bass_kernel_spmd(nc, in_maps, core_ids=list(range(NCORES)))
    return assemble(res.results)


# revision 18
# speedup vs baseline: 1.0340x; 1.0052x over previous
"""SAGAN-style attention block (nn_AttentionBlock) on 8 Trainium2 NeuronCores.

Math (per batch b):
    q = wq @ x + bq            [C8, N]
    k = wk @ x + bk            [C8, N]
    v = wv @ x + bv            [C,  N]
    S[n, m]  = sum_o q[o,n] k[o,m]
    attn     = softmax_m(S)
    out[c,n] = sum_m v[c,m] attn[n,m]
    y        = gamma * out + x

Sharding: 8 cores = 4 batches x 2 halves of the n (query-row) axis.

Design notes (v4, all from trace evidence):
  - q/k path in fp16 (bf16 there costs 1.5e-2 rel err via exp amplification;
    fp16 gives 1.8e-3).  P=exp(S) and v^T in bf16 (P overflows fp16).
  - wq/wk host-tiled 4x across PE row-groups so the projection replicates
    q/k into all four 32-row partition groups; the quadrant-packed QK^T
    matmuls (tile_position=(32g,0), concurrent on the PE) slice them
    directly - no packing DMAs.
  - gamma folded into wv on the host; gamma*bv folded into the xq residual
    input on the host.
  - All big SBUF tensors are split into per-chunk tiles: Tile tracks
    dependencies per tile, so with one monolithic xh tile the first
    k-projection waited for ALL xh DMAs (~24us).  Chunked, the
    QK/exp stream starts as soon as chunk 0 lands.
  - softmax denominator: per-slot [128,2048] bf16 running sum on DVE,
    folded 4->1, partition-summed+broadcast by gpsimd.partition_all_reduce,
    inverted by the fast custom-DVE reciprocal (1-cycle/elem; the [1,512]
    iterative reciprocal cost 4us/block in v1).
  - PSUM: 4 banks S^T (single buffer) + 2x2 accumulator banks alternating
    between blocks so block nb+1's AV matmuls overlap block nb's tail.
  - fp32 residual xq is late-loaded per block (gpsimd queue) so its 2MB
    does not compete with the startup-critical fp16 loads.
"""

import sys

sys.path.insert(0, "/opt/trn_rl_repo")

import numpy as np  # noqa: E402

B, C, HH, WW = 4, 256, 64, 64
N = HH * WW  # 4096
C8 = C // 8  # 32
P = 128
CT = C // P  # 2 channel tiles
NQ = N // 2  # 2048 query rows per core
NBLK = 512  # n-block (query columns per block)
NBLKS = NQ // NBLK  # 4
MT = N // P  # 32 m-tiles (key/value positions)
GRP = 4  # m-tiles per S^T psum slot
NSLOT = MT // GRP  # 8 slots per block
CHUNK = 512
NCHUNKS = N // CHUNK  # 8
QCHUNKS = NQ // CHUNK  # 4
NCORES = 8

_prog = None


def _build(debug_taps=False):
    import concourse.bacc as bacc
    import concourse.bass_isa as bass_isa
    import concourse.mybir as mybir
    import concourse.tile as tile

    f32 = mybir.dt.float32
    f16 = mybir.dt.float16
    bf16 = mybir.dt.bfloat16
    AluAdd = mybir.AluOpType.add
    Exp = mybir.ActivationFunctionType.Exp
    RAdd = bass_isa.ReduceOp.add

    nc = bacc.Bacc("TRN2", target_bir_lowering=False, debug=False)

    dbg = {}
    if debug_taps:
        dbg["dacc"] = nc.dram_tensor("dbg_dacc", [P, GRP * NBLK], bf16, kind="ExternalOutput")
        dbg["dbc"] = nc.dram_tensor("dbg_dbc", [P, NBLK], f32, kind="ExternalOutput")

    xh_d = nc.dram_tensor("xh", [C, N], f16, kind="ExternalInput")
    xt_d = nc.dram_tensor("xt", [N, C], bf16, kind="ExternalInput")
    xqh_d = nc.dram_tensor("xqh", [C, NQ], f16, kind="ExternalInput")
    xq_d = nc.dram_tensor("xq", [C, NQ], f32, kind="ExternalInput")
    wqt4_d = nc.dram_tensor("wqt4", [C, P], f16, kind="ExternalInput")
    wkt4_d = nc.dram_tensor("wkt4", [C, P], f16, kind="ExternalInput")
    wvt_d = nc.dram_tensor("wvt", [C, C], bf16, kind="ExternalInput")
    bq4_d = nc.dram_tensor("bq4", [P], f32, kind="ExternalInput")
    bk4_d = nc.dram_tensor("bk4", [P], f32, kind="ExternalInput")
    out_d = nc.dram_tensor("out", [C, NQ], f32, kind="ExternalOutput")

    with tile.TileContext(nc) as tc:
        with (
            tc.tile_pool(name="const", bufs=1) as const,
            tc.tile_pool(name="big", bufs=1) as big,
        ):
            # per-chunk tiles => fine-grained DMA->compute dependencies
            xh_c = [big.tile([P, CT, CHUNK], f16, name=f"xh{i}") for i in range(NCHUNKS)]
            xqh_c = [big.tile([P, CT, CHUNK], f16, name=f"xqh{i}") for i in range(QCHUNKS)]
            xq_c = [big.tile([P, CT, NBLK], f32, name=f"xq{i}") for i in range(NBLKS)]
            k_c = [big.tile([P, CHUNK], f16, name=f"k{i}") for i in range(NCHUNKS)]
            q_c = [big.tile([P, NBLK], f16, name=f"q{i}") for i in range(NBLKS)]
            xt_c = [big.tile([P, GRP, C], bf16, name=f"xt{i}") for i in range(NSLOT)]

            wqt4 = const.tile([P, CT, P], f16)
            wkt4 = const.tile([P, CT, P], f16)
            wvt = const.tile([P, CT, C], bf16)
            bq4 = const.tile([P, 1], f32)
            bk4 = const.tile([P, 1], f32)

            xh_r = xh_d.ap().rearrange("(t p) n -> p t n", p=P)
            xt_r = xt_d.ap().rearrange("(m p) c -> p m c", p=P)
            xqh_r = xqh_d.ap().rearrange("(t p) n -> p t n", p=P)
            xq_r = xq_d.ap().rearrange("(t p) n -> p t n", p=P)
            out_r = out_d.ap().rearrange("(t p) n -> p t n", p=P)

            # sync queue: k-path weights, then xh (k-proj) and xt (AV lhsT)
            # chunks interleaved so both pipelines start early
            nc.sync.dma_start(out=wkt4, in_=wkt4_d.ap().rearrange("(t p) o -> p t o", p=P))
            nc.sync.dma_start(out=bk4, in_=bk4_d.ap()[:, None])
            for i in range(NCHUNKS):
                sl = slice(i * CHUNK, (i + 1) * CHUNK)
                nc.sync.dma_start(out=xh_c[i], in_=xh_r[:, :, sl])
                nc.sync.dma_start(out=xt_c[i], in_=xt_r[:, GRP * i:GRP * (i + 1), :])
            # scalar queue in parallel: q-path weights, xqh, v weights
            nc.scalar.dma_start(out=wqt4, in_=wqt4_d.ap().rearrange("(t p) o -> p t o", p=P))
            nc.scalar.dma_start(out=bq4, in_=bq4_d.ap()[:, None])
            nc.scalar.dma_start(out=xqh_c[0], in_=xqh_r[:, :, 0:CHUNK])
            nc.scalar.dma_start(out=wvt, in_=wvt_d.ap().rearrange("(t p) o -> p t o", p=P))
            for i in range(1, QCHUNKS):
                sl = slice(i * CHUNK, (i + 1) * CHUNK)
                nc.scalar.dma_start(out=xqh_c[i], in_=xqh_r[:, :, sl])

            # ---- fused projection + attention ----
            # PSUM budget: during block 0 the projection pool (2 banks)
            # coexists with S^T (4) and block-0 accumulators (2) = 8; the
            # blocks-1..3 accumulator pool takes the projection banks after.
            with (
                tc.tile_pool(name="st_ps", bufs=1, space="PSUM") as stp,
                tc.tile_pool(name="acc0_ps", bufs=1, space="PSUM") as accp0,
                tc.tile_pool(name="ptp", bufs=3) as ptp,
                tc.tile_pool(name="dap", bufs=2) as dap,
                tc.tile_pool(name="dnp", bufs=2) as dnp,
                tc.tile_pool(name="finp", bufs=4) as finp,
            ):
                bstate = {}

                def emit_av(nb, mg, pt):
                    accs, dacc, nb_ = bstate[nb]
                    for i in range(GRP):
                        mt = GRP * mg + i
                        for cc in range(CT):
                            nc.tensor.matmul(
                                accs[cc],
                                lhsT=xt_c[mg][:, i, cc * P:(cc + 1) * P],
                                rhs=pt[:, i, :],
                                start=(mt == 0),
                                stop=(mt == MT - 1),
                            )
                    # denominator partial: one 2048-elem bf16 add per slot
                    if mg == 0:
                        nc.vector.tensor_copy(out=dacc, in_=pt)
                    else:
                        nc.vector.tensor_tensor(dacc, dacc, pt, AluAdd)

                def emit_tail(nb):
                    accs, dacc, nb_ = bstate.pop(nb)
                    nsl = slice(nb * NBLK, (nb + 1) * NBLK)
                    # drain Z = x @ P^T to SBUF, then out = wv_g @ Z reusing
                    # the same accumulator banks (WAR dep via Tile)
                    zsb = finp.tile([P, CT, NBLK], bf16, tag="zsb", name="zsb")
                    for cc in range(CT):
                        nc.vector.tensor_copy(out=zsb[:, cc, :], in_=accs[cc])
                    for co in range(CT):
                        for ci in range(CT):
                            nc.tensor.matmul(
                                accs[co],
                                lhsT=wvt[:, ci, co * P:(co + 1) * P],
                                rhs=zsb[:, ci, :],
                                start=(ci == 0),
                                stop=(ci == CT - 1),
                            )
                    d2 = dnp.tile([P, 2, NBLK], bf16, tag="d2", name="d2")
                    nc.vector.tensor_tensor(d2, dacc[:, 0:2, :], dacc[:, 2:4, :], AluAdd)
                    d1 = dnp.tile([P, NBLK], bf16, tag="d1", name="d1")
                    nc.vector.tensor_tensor(d1, d2[:, 0, :], d2[:, 1, :], AluAdd)
                    # sum over partitions, result broadcast to all partitions
                    dbc = dnp.tile([P, NBLK], f32, tag="dbc", name="dbc")
                    nc.gpsimd.partition_all_reduce(dbc, d1, channels=P, reduce_op=RAdd)
                    rec = dnp.tile([P, NBLK], f32, tag="rec", name="rec")
                    nc.vector.reciprocal_approx_fast(rec, dbc)
                    # gated late-load of the NEXT block's fp32 residual slice
                    # (gpsimd FIFO: issues only once this all_reduce is done)
                    if nb + 1 < NBLKS:
                        nn_ = slice((nb + 1) * NBLK, (nb + 2) * NBLK)
                        nc.gpsimd.dma_start(out=xq_c[nb + 1], in_=xq_r[:, :, nn_])
                    if debug_taps and nb == 0:
                        nc.sync.dma_start(out=dbg["dacc"].ap().rearrange("p (g n) -> p g n", g=GRP), in_=dacc)
                        nc.sync.dma_start(out=dbg["dbc"].ap(), in_=dbc)
                    for cc in range(CT):
                        fin = finp.tile([P, NBLK], f32, tag="fin", name="fin")
                        nc.vector.tensor_mul(out=fin, in0=accs[cc], in1=rec)
                        nc.vector.tensor_add(out=fin, in0=fin, in1=xq_c[nb_][:, cc, :])
                        nc.sync.dma_start(out=out_r[:, cc, nsl], in_=fin)

                def emit_slot(nb, mg):
                    st = stp.tile([P, GRP, NBLK], f32, tag="st", name="st")
                    for g in range(GRP):
                        nc.tensor.matmul(
                            st[:, g, :],
                            lhsT=k_c[mg][32 * g:32 * g + 32, g * P:(g + 1) * P],
                            rhs=q_c[nb][32 * g:32 * g + 32, :],
                            start=True,
                            stop=True,
                            tile_position=(32 * g, 0),
                        )
                    pt = ptp.tile([P, GRP, NBLK], bf16, tag="pt", name="pt")
                    nc.scalar.activation(out=pt, in_=st, func=Exp)
                    return pt

                prev = None

                def pump(nb, mg, pt):
                    nonlocal prev
                    if prev is not None:
                        pnb, pmg, ppt = prev
                        emit_av(pnb, pmg, ppt)
                        if pmg == NSLOT - 1:
                            emit_tail(pnb)
                    prev = (nb, mg, pt)

                def new_block(nb, pool):
                    a0 = pool.tile([P, NBLK], f32, tag="o0", name="a0")
                    a1 = pool.tile([P, NBLK], f32, tag="o1", name="a1")
                    dacc = dap.tile([P, GRP, NBLK], bf16, tag="da", name="dacc")
                    bstate[nb] = ([a0, a1], dacc, nb)

                # --- block 0: k/q projections stream between its slots ---
                with tc.tile_pool(name="pa", bufs=2, space="PSUM") as pap:
                    def proj_chunk(dst, w4, bcol, src, name):
                        pp = pap.tile([P, CHUNK], f32, tag="pj", name=name)
                        for t in range(CT):
                            nc.tensor.matmul(
                                pp, lhsT=w4[:, t, :], rhs=src[:, t, :],
                                start=(t == 0), stop=(t == CT - 1),
                            )
                        # fused drain + per-partition bias add + f16 cast
                        nc.vector.tensor_scalar_add(dst, pp, bcol)

                    nc.gpsimd.dma_start(out=xq_c[0], in_=xq_r[:, :, 0:NBLK])
                    proj_chunk(k_c[0], wkt4, bk4, xh_c[0], "kp")
                    proj_chunk(q_c[0], wqt4, bq4, xqh_c[0], "qp")
                    new_block(0, accp0)
                    for mg in range(NSLOT):
                        pt = emit_slot(0, mg)
                        if mg < NSLOT - 1:
                            proj_chunk(k_c[mg + 1], wkt4, bk4, xh_c[mg + 1], "kp")
                        if mg < QCHUNKS - 1:
                            proj_chunk(q_c[mg + 1], wqt4, bq4, xqh_c[mg + 1], "qp")
                        pump(0, mg, pt)

                # --- blocks 1..3 on the second accumulator pool ---
                with tc.tile_pool(name="acc_ps", bufs=1, space="PSUM") as accp:
                    for nb in range(1, NBLKS):
                        new_block(nb, accp)
                        for mg in range(NSLOT):
                            pt = emit_slot(nb, mg)
                            pump(nb, mg, pt)
                    pnb, pmg, ppt = prev
                    emit_av(pnb, pmg, ppt)
                    emit_tail(pnb)

    nc.compile()
    return nc


def _get_prog():
    global _prog
    if _prog is None:
        _prog = _build()
    return _prog


def make_in_maps(inputs):
    x = np.ascontiguousarray(inputs["x"], dtype=np.float32).reshape(B, C, N)
    gamma = float(np.asarray(inputs["gamma"], np.float32).reshape(()))
    wq = np.asarray(inputs["wq"], np.float32)
    wk = np.asarray(inputs["wk"], np.float32)
    wv = np.asarray(inputs["wv"], np.float32)
    bq = np.asarray(inputs["bq"], np.float32)
    bk = np.asarray(inputs["bk"], np.float32)
    bv = np.asarray(inputs["bv"], np.float32)

    wqt4 = np.ascontiguousarray(np.tile(wq.T, (1, 4)).astype(np.float16))  # [C,128]
    wkt4 = np.ascontiguousarray(np.tile(wk.T, (1, 4)).astype(np.float16))
    import ml_dtypes

    wvt = np.ascontiguousarray((gamma * wv.T).astype(ml_dtypes.bfloat16))  # [C,C]
    bq4 = np.ascontiguousarray(np.tile(bq, 4).astype(np.float32))  # [128]
    bk4 = np.ascontiguousarray(np.tile(bk, 4).astype(np.float32))
    gbv = (gamma * bv).astype(np.float32)[:, None]  # [C,1]

    x_f16 = x.astype(np.float16)  # [B,C,N]
    x_t = np.ascontiguousarray(x.transpose(0, 2, 1)).astype(ml_dtypes.bfloat16)  # [B,N,C]
    in_maps = []
    for core in range(NCORES):
        b, h = divmod(core, 2)
        xq = x[b][:, h * NQ:(h + 1) * NQ] + gbv
        in_maps.append(
            {
                "xh": x_f16[b],
                "xt": x_t[b],
                "xqh": np.ascontiguousarray(x_f16[b][:, h * NQ:(h + 1) * NQ]),
                "xq": np.ascontiguousarray(xq, dtype=np.float32),
                "wqt4": wqt4,
                "wkt4": wkt4,
                "wvt": wvt,
                "bq4": bq4,
                "bk4": bk4,
            }
        )
    return in_maps


def assemble(results):
    out = np.empty((B, C, N), np.float32)
    for core in range(NCORES):
        b, h = divmod(core, 2)
        out[b][:, h * NQ:(h + 1) * NQ] = results[core]["out"]
    return out.reshape(B, C, HH, WW)


def kernel(**inputs):
    from concourse.bass_utils import run_bass_kernel_spmd

    nc = _get_prog()
    in_maps = make_in_maps(inputs)
    res = run_bass_kernel_spmd(nc, in_maps, core_ids=list(range(NCORES)))
    return assemble(res.results)


# revision 19
# speedup vs baseline: 1.1187x; 1.0820x over previous
"""SAGAN-style attention block (nn_AttentionBlock) on 8 Trainium2 NeuronCores.

Math (per batch b):
    q = wq @ x + bq            [C8, N]
    k = wk @ x + bk            [C8, N]
    v = wv @ x + bv            [C,  N]
    S[n, m]  = sum_o q[o,n] k[o,m]
    attn     = softmax_m(S)
    out[c,n] = sum_m v[c,m] attn[n,m]
    y        = gamma * out + x

Sharding: 8 cores = 4 batches x 2 halves of the n (query-row) axis.

Design notes (v4, all from trace evidence):
  - q/k path in fp16 (bf16 there costs 1.5e-2 rel err via exp amplification;
    fp16 gives 1.8e-3).  P=exp(S) and v^T in bf16 (P overflows fp16).
  - wq/wk host-tiled 4x across PE row-groups so the projection replicates
    q/k into all four 32-row partition groups; the quadrant-packed QK^T
    matmuls (tile_position=(32g,0), concurrent on the PE) slice them
    directly - no packing DMAs.
  - gamma folded into wv on the host; gamma*bv folded into the xq residual
    input on the host.
  - All big SBUF tensors are split into per-chunk tiles: Tile tracks
    dependencies per tile, so with one monolithic xh tile the first
    k-projection waited for ALL xh DMAs (~24us).  Chunked, the
    QK/exp stream starts as soon as chunk 0 lands.
  - softmax denominator: per-slot [128,2048] bf16 running sum on DVE,
    folded 4->1, partition-summed+broadcast by gpsimd.partition_all_reduce,
    inverted by the fast custom-DVE reciprocal (1-cycle/elem; the [1,512]
    iterative reciprocal cost 4us/block in v1).
  - PSUM: 4 banks S^T (single buffer) + 2x2 accumulator banks alternating
    between blocks so block nb+1's AV matmuls overlap block nb's tail.
  - fp32 residual xq is late-loaded per block (gpsimd queue) so its 2MB
    does not compete with the startup-critical fp16 loads.
"""

import sys

sys.path.insert(0, "/opt/trn_rl_repo")

import numpy as np  # noqa: E402

B, C, HH, WW = 4, 256, 64, 64
N = HH * WW  # 4096
C8 = C // 8  # 32
P = 128
CT = C // P  # 2 channel tiles
NQ = N // 2  # 2048 query rows per core
NBLK = 512  # n-block (query columns per block)
NBLKS = NQ // NBLK  # 4
MT = N // P  # 32 m-tiles (key/value positions)
GRP = 4  # m-tiles per S^T psum slot
NSLOT = MT // GRP  # 8 slots per block
CHUNK = 512
NCHUNKS = N // CHUNK  # 8
QCHUNKS = NQ // CHUNK  # 4
NCORES = 8

_prog = None


def _build(debug_taps=False):
    import concourse.bacc as bacc
    import concourse.bass_isa as bass_isa
    import concourse.mybir as mybir
    import concourse.tile as tile

    f32 = mybir.dt.float32
    f16 = mybir.dt.float16
    bf16 = mybir.dt.bfloat16
    AluAdd = mybir.AluOpType.add
    Exp = mybir.ActivationFunctionType.Exp
    RAdd = bass_isa.ReduceOp.add

    nc = bacc.Bacc("TRN2", target_bir_lowering=False, debug=False)

    dbg = {}
    if debug_taps:
        dbg["dacc"] = nc.dram_tensor("dbg_dacc", [P, GRP * NBLK], bf16, kind="ExternalOutput")
        dbg["dbc"] = nc.dram_tensor("dbg_dbc", [P, NBLK], f32, kind="ExternalOutput")

    xh_d = nc.dram_tensor("xh", [C, N], f16, kind="ExternalInput")
    xt_d = nc.dram_tensor("xt", [N, C], bf16, kind="ExternalInput")
    xqh_d = nc.dram_tensor("xqh", [C, NQ], f16, kind="ExternalInput")
    xq_d = nc.dram_tensor("xq", [C, NQ], f32, kind="ExternalInput")
    wqt4_d = nc.dram_tensor("wqt4", [C, P], f16, kind="ExternalInput")
    wkt4_d = nc.dram_tensor("wkt4", [C, P], f16, kind="ExternalInput")
    wvt_d = nc.dram_tensor("wvt", [C, C], bf16, kind="ExternalInput")
    bq4_d = nc.dram_tensor("bq4", [P], f32, kind="ExternalInput")
    bk4_d = nc.dram_tensor("bk4", [P], f32, kind="ExternalInput")
    out_d = nc.dram_tensor("out", [C, NQ], f32, kind="ExternalOutput")

    with tile.TileContext(nc) as tc:
        with (
            tc.tile_pool(name="const", bufs=1) as const,
            tc.tile_pool(name="big", bufs=1) as big,
        ):
            # per-chunk tiles => fine-grained DMA->compute dependencies
            xh_c = [big.tile([P, CT, CHUNK], f16, name=f"xh{i}") for i in range(NCHUNKS)]
            xqh_c = [big.tile([P, CT, CHUNK], f16, name=f"xqh{i}") for i in range(QCHUNKS)]
            xq_c = [big.tile([P, CT, NBLK], f32, name=f"xq{i}") for i in range(NBLKS)]
            k_c = [big.tile([P, CHUNK], f16, name=f"k{i}") for i in range(NCHUNKS)]
            q_c = [big.tile([P, NBLK], f16, name=f"q{i}") for i in range(NBLKS)]
            xt_c = [big.tile([P, GRP, C], bf16, name=f"xt{i}") for i in range(NSLOT)]

            wqt4 = const.tile([P, CT, P], f16)
            wkt4 = const.tile([P, CT, P], f16)
            wvt = const.tile([P, CT, C], bf16)
            bq4 = const.tile([P, 1], f32)
            bk4 = const.tile([P, 1], f32)

            xh_r = xh_d.ap().rearrange("(t p) n -> p t n", p=P)
            xt_r = xt_d.ap().rearrange("(m p) c -> p m c", p=P)
            xqh_r = xqh_d.ap().rearrange("(t p) n -> p t n", p=P)
            xq_r = xq_d.ap().rearrange("(t p) n -> p t n", p=P)
            out_r = out_d.ap().rearrange("(t p) n -> p t n", p=P)

            # sync queue: k-path weights, then xh (k-proj) and xt (AV lhsT)
            # chunks interleaved so both pipelines start early
            nc.sync.dma_start(out=wkt4, in_=wkt4_d.ap().rearrange("(t p) o -> p t o", p=P))
            nc.sync.dma_start(out=bk4, in_=bk4_d.ap()[:, None])
            for i in range(NCHUNKS):
                sl = slice(i * CHUNK, (i + 1) * CHUNK)
                nc.sync.dma_start(out=xh_c[i], in_=xh_r[:, :, sl])
            for i in range(NCHUNKS):
                nc.sync.dma_start(out=xt_c[i], in_=xt_r[:, GRP * i:GRP * (i + 1), :])
            # scalar queue in parallel: q-path weights, xqh, v weights last
            nc.scalar.dma_start(out=wqt4, in_=wqt4_d.ap().rearrange("(t p) o -> p t o", p=P))
            nc.scalar.dma_start(out=bq4, in_=bq4_d.ap()[:, None])
            for i in range(QCHUNKS):
                sl = slice(i * CHUNK, (i + 1) * CHUNK)
                nc.scalar.dma_start(out=xqh_c[i], in_=xqh_r[:, :, sl])
            nc.scalar.dma_start(out=wvt, in_=wvt_d.ap().rearrange("(t p) o -> p t o", p=P))

            # ---- fused projection + attention ----
            # PSUM budget: during block 0 the projection pool (2 banks)
            # coexists with S^T (4) and block-0 accumulators (2) = 8; the
            # blocks-1..3 accumulator pool takes the projection banks after.
            with (
                tc.tile_pool(name="st_ps", bufs=1, space="PSUM") as stp,
                tc.tile_pool(name="ptp", bufs=3) as ptp,
                tc.tile_pool(name="dap", bufs=2) as dap,
                tc.tile_pool(name="dnp", bufs=2) as dnp,
                tc.tile_pool(name="finp", bufs=4) as finp,
            ):
                bstate = {}

                def emit_av(nb, mg, pt):
                    accs, dacc, nb_ = bstate[nb]
                    for i in range(GRP):
                        mt = GRP * mg + i
                        for cc in range(CT):
                            nc.tensor.matmul(
                                accs[cc],
                                lhsT=xt_c[mg][:, i, cc * P:(cc + 1) * P],
                                rhs=pt[:, i, :],
                                start=(mt == 0),
                                stop=(mt == MT - 1),
                            )
                    # denominator partial: one 2048-elem bf16 add per slot
                    if mg == 0:
                        nc.vector.tensor_copy(out=dacc, in_=pt)
                    else:
                        nc.vector.tensor_tensor(dacc, dacc, pt, AluAdd)

                def emit_tail(nb):
                    accs, dacc, nb_ = bstate.pop(nb)
                    nsl = slice(nb * NBLK, (nb + 1) * NBLK)
                    # drain Z = x @ P^T to SBUF, then out = wv_g @ Z reusing
                    # the same accumulator banks (WAR dep via Tile)
                    zsb = finp.tile([P, CT, NBLK], bf16, tag="zsb", name="zsb")
                    for cc in range(CT):
                        nc.vector.tensor_copy(out=zsb[:, cc, :], in_=accs[cc])
                    for co in range(CT):
                        for ci in range(CT):
                            nc.tensor.matmul(
                                accs[co],
                                lhsT=wvt[:, ci, co * P:(co + 1) * P],
                                rhs=zsb[:, ci, :],
                                start=(ci == 0),
                                stop=(ci == CT - 1),
                            )
                    d2 = dnp.tile([P, 2, NBLK], bf16, tag="d2", name="d2")
                    nc.vector.tensor_tensor(d2, dacc[:, 0:2, :], dacc[:, 2:4, :], AluAdd)
                    d1 = dnp.tile([P, NBLK], bf16, tag="d1", name="d1")
                    nc.vector.tensor_tensor(d1, d2[:, 0, :], d2[:, 1, :], AluAdd)
                    # sum over partitions, result broadcast to all partitions
                    dbc = dnp.tile([P, NBLK], f32, tag="dbc", name="dbc")
                    nc.gpsimd.partition_all_reduce(dbc, d1, channels=P, reduce_op=RAdd)
                    rec = dnp.tile([P, NBLK], f32, tag="rec", name="rec")
                    nc.vector.reciprocal_approx_fast(rec, dbc)
                    # gated late-load of the NEXT block's fp32 residual slice
                    # (gpsimd FIFO: issues only once this all_reduce is done)
                    if nb + 1 < NBLKS:
                        nn_ = slice((nb + 1) * NBLK, (nb + 2) * NBLK)
                        nc.gpsimd.dma_start(out=xq_c[nb + 1], in_=xq_r[:, :, nn_])
                    if debug_taps and nb == 0:
                        nc.sync.dma_start(out=dbg["dacc"].ap().rearrange("p (g n) -> p g n", g=GRP), in_=dacc)
                        nc.sync.dma_start(out=dbg["dbc"].ap(), in_=dbc)
                    for cc in range(CT):
                        fin = finp.tile([P, NBLK], f32, tag="fin", name="fin")
                        nc.vector.tensor_mul(out=fin, in0=accs[cc], in1=rec)
                        nc.vector.tensor_add(out=fin, in0=fin, in1=xq_c[nb_][:, cc, :])
                        nc.sync.dma_start(out=out_r[:, cc, nsl], in_=fin)

                def emit_slot(nb, mg):
                    st = stp.tile([P, GRP, NBLK], f32, tag="st", name="st")
                    for g in range(GRP):
                        nc.tensor.matmul(
                            st[:, g, :],
                            lhsT=k_c[mg][32 * g:32 * g + 32, g * P:(g + 1) * P],
                            rhs=q_c[nb][32 * g:32 * g + 32, :],
                            start=True,
                            stop=True,
                            tile_position=(32 * g, 0),
                        )
                    pt = ptp.tile([P, GRP, NBLK], bf16, tag="pt", name="pt")
                    nc.scalar.activation(out=pt, in_=st, func=Exp)
                    return pt

                prev = None

                def pump(nb, mg, pt):
                    nonlocal prev
                    if prev is not None:
                        pnb, pmg, ppt = prev
                        emit_av(pnb, pmg, ppt)
                        if pmg == NSLOT - 1:
                            emit_tail(pnb)
                    prev = (nb, mg, pt)

                def new_block(nb, pool):
                    a0 = pool.tile([P, NBLK], f32, tag="o0", name="a0")
                    a1 = pool.tile([P, NBLK], f32, tag="o1", name="a1")
                    dacc = dap.tile([P, GRP, NBLK], bf16, tag="da", name="dacc")
                    bstate[nb] = ([a0, a1], dacc, nb)

                # --- k/q projections (their psum banks free before AV) ---
                with tc.tile_pool(name="pa", bufs=2, space="PSUM") as pap:
                    def proj_chunk(dst, w4, bcol, src, name):
                        pp = pap.tile([P, CHUNK], f32, tag="pj", name=name)
                        for t in range(CT):
                            nc.tensor.matmul(
                                pp, lhsT=w4[:, t, :], rhs=src[:, t, :],
                                start=(t == 0), stop=(t == CT - 1),
                            )
                        # fused drain + per-partition bias add + f16 cast
                        nc.vector.tensor_scalar_add(dst, pp, bcol)

                    nc.gpsimd.dma_start(out=xq_c[0], in_=xq_r[:, :, 0:NBLK])
                    for ch in range(NCHUNKS):
                        proj_chunk(k_c[ch], wkt4, bk4, xh_c[ch], "kp")
                        if ch < QCHUNKS:
                            proj_chunk(q_c[ch], wqt4, bq4, xqh_c[ch], "qp")

                # --- attention blocks, accumulators alternate (bufs=2) ---
                with tc.tile_pool(name="acc_ps", bufs=2, space="PSUM") as accp:
                    for nb in range(NBLKS):
                        new_block(nb, accp)
                        for mg in range(NSLOT):
                            pt = emit_slot(nb, mg)
                            pump(nb, mg, pt)
                    pnb, pmg, ppt = prev
                    emit_av(pnb, pmg, ppt)
                    emit_tail(pnb)

    nc.compile()
    return nc


def _get_prog():
    global _prog
    if _prog is None:
        _prog = _build()
    return _prog


def make_in_maps(inputs):
    x = np.ascontiguousarray(inputs["x"], dtype=np.float32).reshape(B, C, N)
    gamma = float(np.asarray(inputs["gamma"], np.float32).reshape(()))
    wq = np.asarray(inputs["wq"], np.float32)
    wk = np.asarray(inputs["wk"], np.float32)
    wv = np.asarray(inputs["wv"], np.float32)
    bq = np.asarray(inputs["bq"], np.float32)
    bk = np.asarray(inputs["bk"], np.float32)
    bv = np.asarray(inputs["bv"], np.float32)

    wqt4 = np.ascontiguousarray(np.tile(wq.T, (1, 4)).astype(np.float16))  # [C,128]
    wkt4 = np.ascontiguousarray(np.tile(wk.T, (1, 4)).astype(np.float16))
    import ml_dtypes

    wvt = np.ascontiguousarray((gamma * wv.T).astype(ml_dtypes.bfloat16))  # [C,C]
    bq4 = np.ascontiguousarray(np.tile(bq, 4).astype(np.float32))  # [128]
    bk4 = np.ascontiguousarray(np.tile(bk, 4).astype(np.float32))
    gbv = (gamma * bv).astype(np.float32)[:, None]  # [C,1]

    x_f16 = x.astype(np.float16)  # [B,C,N]
    x_t = np.ascontiguousarray(x.transpose(0, 2, 1)).astype(ml_dtypes.bfloat16)  # [B,N,C]
    in_maps = []
    for core in range(NCORES):
        b, h = divmod(core, 2)
        xq = x[b][:, h * NQ:(h + 1) * NQ] + gbv
        in_maps.append(
            {
                "xh": x_f16[b],
                "xt": x_t[b],
                "xqh": np.ascontiguousarray(x_f16[b][:, h * NQ:(h + 1) * NQ]),
                "xq": np.ascontiguousarray(xq, dtype=np.float32),
                "wqt4": wqt4,
                "wkt4": wkt4,
                "wvt": wvt,
                "bq4": bq4,
                "bk4": bk4,
            }
        )
    return in_maps


def assemble(results):
    out = np.empty((B, C, N), np.float32)
    for core in range(NCORES):
        b, h = divmod(core, 2)
        out[b][:, h * NQ:(h + 1) * NQ] = results[core]["out"]
    return out.reshape(B, C, HH, WW)


def kernel(**inputs):
    from concourse.bass_utils import run_bass_kernel_spmd

    nc = _get_prog()
    in_maps = make_in_maps(inputs)
    res = run_bass_kernel_spmd(nc, in_maps, core_ids=list(range(NCORES)))
    return assemble(res.results)
